# revision 12
# baseline (speedup 1.0000x reference)
"""Trainium2 Bass kernel for nn_BackboneRaindrop (GNN message passing + transformer).

Self-contained: accepts FULL inputs, shards batch across 8 NeuronCores,
returns the FULL output tuple (output [L,B,D] f32, mask [B,L] bool).
"""
import os
import sys

for _p in ("/opt/trn_rl_repo",):
    if _p not in sys.path:
        sys.path.insert(0, _p)

import numpy as np
import ml_dtypes

import concourse.bass as bass
import concourse.tile as tile
from concourse import bacc, mybir
from concourse.bass_utils import run_bass_kernel_spmd

F32 = mybir.dt.float32
BF16 = mybir.dt.bfloat16
AF = mybir.ActivationFunctionType
MUL = mybir.AluOpType.mult
ADD = mybir.AluOpType.add

BF = ml_dtypes.bfloat16

# model config
B, F_, L = 64, 128, 256
D_OB = 8
D_PE = 16
N_HEADS = 16
N_LAYERS = 2
D_FFN = 2048
C = L * D_OB            # 2048
D = F_ * D_OB + D_PE    # 1040
HD = D // N_HEADS       # 65
LN_EPS = 1e-5

NCORES = 8
BL = B // NCORES        # 8 samples per core
T = BL * L              # 2048 tokens per core, token = b*256 + l
NMT = 9                 # stream m'-tiles (8x128 + 1x16)
KT_GNN = 16

STAGES = int(os.environ.get("KSTAGES", "3"))  # 1=GNN only, 2=+layer0, 3=full

# feature permutation: stream row m' = j*128 + f  <->  original d = f*8 + j
_mp = np.arange(1024)
PERM_M = np.concatenate([(_mp % 128) * 8 + (_mp // 128), np.arange(1024, 1040)])
# GNN contraction order c' = j*256 + l  <->  original c = l*8 + j
_cp = np.arange(2048)
PERM_C = (_cp % 256) * 8 + (_cp // 256)

# q/k main/left row maps (within a 1040-row q or k block)
_qm = np.arange(1024)
QMAIN_ROWS = (_qm // 64) * 65 + (_qm % 64)   # head h dims 0..63
QLEFT_ROWS = np.arange(16) * 65 + 64         # head h dim 64
# e96 map for Wo contraction (head pitch 96)
_e96 = np.arange(12 * 128)
E96_H = _e96 // 96
E96_R = _e96 % 96
E96_VALID = (E96_R < 65) & (E96_H < 16)


def _to_bf(x):
    return np.ascontiguousarray(x.astype(np.float32).astype(BF))


def _f32(x):
    return np.ascontiguousarray(x.astype(np.float32))


def _range_reduce(x64):
    two_pi = 2.0 * np.pi
    return (x64 - two_pi * np.round(x64 / two_pi)).astype(np.float32)


def prep_weights(w):
    """Host-side weight layout prep (shared across cores). Returns dict name->np array."""
    out = {}
    Wv1 = _f32(w["Wv1"]); Wv2 = _f32(w["Wv2"])
    # m1: lhsT[kt, p, d1] = Wv1[d1, PERM_C[kt*128+p]]
    out["wv1t"] = _to_bf(Wv1[:, PERM_C].T.reshape(KT_GNN, 128, 2048))
    # m2: lhsT is x1; rhs[kt, p, d2''] = Wv2[PERM_C[d2''], kt*128+p]
    out["wv2t"] = _to_bf(Wv2[PERM_C, :].T.reshape(KT_GNN, 128, 2048))
    out["bv1pp"] = _f32(w["bv1"]).reshape(16, 128).T.copy()            # [128,16] per-partition
    out["bv2row"] = _to_bf(w["bv2"][PERM_C].reshape(1, 2048))
    r = _f32(w["R_u"]).reshape(F_, D_OB)                               # r[f, j]
    out["r8"] = _to_bf(np.tile(r.T, (1, BL)))                          # [8, 1024]

    for i in range(N_LAYERS):
        Wqkv = _f32(w["Wqkv"][i]); bqkv = _f32(w["bqkv"][i])
        Wq, Wk, Wv = Wqkv[:D], Wqkv[D:2*D], Wqkv[2*D:]
        bq, bk, bv = bqkv[:D], bqkv[D:2*D], bqkv[2*D:]
        Wcols = lambda W, rows: W[rows][:, PERM_M].T                   # [1040(m'), nrows]

        def pad_kt(a, width):  # [1040, width] -> [9, 128, width] zero-padded
            full = np.zeros((NMT * 128, width), np.float32)
            full[:1040] = a
            return full.reshape(NMT, 128, width)

        out[f"wq{i}"] = _to_bf(pad_kt(Wcols(Wq, QMAIN_ROWS), 1024))
        out[f"wk{i}"] = _to_bf(pad_kt(Wcols(Wk, QMAIN_ROWS), 1024))
        out[f"wql{i}"] = _to_bf(pad_kt(Wcols(Wq, QLEFT_ROWS), 16))
        out[f"wkl{i}"] = _to_bf(pad_kt(Wcols(Wk, QLEFT_ROWS), 16))
        out[f"wv{i}"] = _to_bf(pad_kt(Wcols(Wv, np.arange(D)), D))
        bq_pp = np.zeros((128, 8), np.float32)
        bk_pp = np.zeros((128, 8), np.float32)
        for qt in range(8):
            bq_pp[:, qt] = bq[QMAIN_ROWS[qt*128:(qt+1)*128]]
            bk_pp[:, qt] = bk[QMAIN_ROWS[qt*128:(qt+1)*128]]
        out[f"bq{i}"] = bq_pp
        out[f"bk{i}"] = bk_pp
        out[f"bql{i}"] = bq[QLEFT_ROWS].reshape(16, 1).copy()
        out[f"bkl{i}"] = bk[QLEFT_ROWS].reshape(16, 1).copy()
        out[f"bvrow{i}"] = _to_bf(bv.reshape(1, D))

        Wo = _f32(w["Wo"][i])                                          # [D, D] out x in
        wo16 = np.zeros((16, 128, 1040), np.float32)
        for h in range(16):
            wo16[h, :65] = Wo[PERM_M][:, h*65:(h+1)*65].T
        out[f"wo{i}"] = _to_bf(wo16)
        out[f"borow{i}"] = _to_bf(w["bo"][i][PERM_M].reshape(1, D))

        Wf1 = _f32(w["Wf1"][i])                                        # [2048, 1040]
        out[f"wf1{i}"] = _to_bf(pad_kt(Wf1[:, PERM_M].T, D_FFN))
        out[f"bf1pp{i}"] = _f32(w["bf1"][i]).reshape(16, 128).T.copy()
        Wf2 = _f32(w["Wf2"][i])                                        # [1040, 2048]
        out[f"wf2{i}"] = _to_bf(Wf2[PERM_M, :].T.reshape(16, 128, 1040))
        out[f"bf2row{i}"] = _to_bf(w["bf2"][i][PERM_M].reshape(1, D))

        for ln in (1, 2):
            s = _f32(w[f"ln{ln}_s"][i])[PERM_M]
            bb = _f32(w[f"ln{ln}_b"][i])[PERM_M]
            s_pp = np.zeros((128, NMT), np.float32)
            b_pp = np.zeros((128, NMT), np.float32)
            for mt in range(NMT):
                n = min(128, 1040 - mt * 128)
                s_pp[:n, mt] = s[mt*128:mt*128+n]
                b_pp[:n, mt] = bb[mt*128:mt*128+n]
            out[f"ln{ln}s{i}"] = s_pp
            out[f"ln{ln}b{i}"] = b_pp
            out[f"ln{ln}srow{i}"] = _to_bf(s.reshape(1, D))
            out[f"ln{ln}brow{i}"] = _to_bf(bb.reshape(1, D))
    return out


def prep_core_inputs(X, timestamps, lengths, core):
    """Per-core activation inputs."""
    bs = core * BL
    Xc = _f32(X[bs:bs+BL])                                             # [8, 128, 256]
    out = {"xt": _to_bf(Xc.transpose(2, 0, 1).reshape(L, BL * F_))}    # [256, 1024]

    ts_scales = (float(L) ** np.linspace(0.0, 1.0, D_PE // 2)).astype(np.float32)
    tloc = _f32(timestamps[bs:bs+BL])                                  # [8, 256]
    tok = tloc.reshape(T)                                              # token = b*256 + l
    scaled = (tok[None, :] / ts_scales[:, None]).astype(np.float32)    # [8, 2048] f32 (match ref)
    out["thsin"] = _range_reduce(scaled.astype(np.float64))
    out["thcos"] = _range_reduce(scaled.astype(np.float64) + np.pi / 2)

    lens = np.asarray(lengths[bs:bs+BL]).reshape(BL).astype(np.int64)
    valid = (np.arange(L)[None, :] < lens[:, None])                    # [8, 256]
    m01 = valid.reshape(T).astype(np.float32).reshape(16, 128).T.copy()  # [128, 16] token tiles
    out["mask01f"] = np.ascontiguousarray(m01)
    out["mask01bf"] = _to_bf(m01)
    return out


# ---------------------------------------------------------------------------
# device kernel build
# ---------------------------------------------------------------------------

def build_nc():
    nc = bacc.Bacc("TRN2", target_bir_lowering=False, debug=False, num_devices=NCORES)
    dt_in = {}

    def din(name, shape, dtype):
        t = nc.dram_tensor(name, list(shape), dtype, kind="ExternalInput")
        dt_in[name] = t
        return t

    # weights (replicated across cores)
    din("wv1t", (KT_GNN, 128, 2048), BF16)
    din("wv2t", (KT_GNN, 128, 2048), BF16)
    din("bv1pp", (128, 16), F32)
    din("bv2row", (1, 2048), BF16)
    din("r8", (8, 1024), BF16)
    for i in range(N_LAYERS):
        din(f"wq{i}", (NMT, 128, 1024), BF16)
        din(f"wk{i}", (NMT, 128, 1024), BF16)
        din(f"wql{i}", (NMT, 128, 16), BF16)
        din(f"wkl{i}", (NMT, 128, 16), BF16)
        din(f"wv{i}", (NMT, 128, D), BF16)
        din(f"bq{i}", (128, 8), F32)
        din(f"bk{i}", (128, 8), F32)
        din(f"bql{i}", (16, 1), F32)
        din(f"bkl{i}", (16, 1), F32)
        din(f"bvrow{i}", (1, D), BF16)
        din(f"wo{i}", (16, 128, D), BF16)
        din(f"borow{i}", (1, D), BF16)
        din(f"wf1{i}", (NMT, 128, D_FFN), BF16)
        din(f"bf1pp{i}", (128, 16), F32)
        din(f"wf2{i}", (16, 128, D), BF16)
        din(f"bf2row{i}", (1, D), BF16)
        for ln in (1, 2):
            din(f"ln{ln}s{i}", (128, NMT), F32)
            din(f"ln{ln}b{i}", (128, NMT), F32)
            din(f"ln{ln}srow{i}", (1, D), BF16)
            din(f"ln{ln}brow{i}", (1, D), BF16)
    # per-core activations
    din("xt", (L, BL * F_), BF16)
    din("thsin", (8, T), F32)
    din("thcos", (8, T), F32)
    din("mask01f", (128, 16), F32)
    din("mask01bf", (128, 16), BF16)

    # DRAM scratch
    xA = nc.dram_tensor("xA", [NMT, 128, T], F32)          # GNN out f32 (stream)
    xAbf = nc.dram_tensor("xAbf", [NMT, 128, T], BF16)
    ao_d = nc.dram_tensor("ao_d", [16, 65, T], BF16)
    x1f_d = nc.dram_tensor("x1f_d", [NMT, 128, T], F32)
    x1bf_d = nc.dram_tensor("x1bf_d", [NMT, 128, T], BF16)
    xB = nc.dram_tensor("xB", [NMT, 128, T], F32)
    xBbf = nc.dram_tensor("xBbf", [NMT, 128, T], BF16)
    out_stream = nc.dram_tensor("out_stream", [NMT, 128, T], F32, kind="ExternalOutput")
    if STAGES == 1:
        xA = out_stream

    inv_sqrt_hd = 1.0 / float(np.sqrt(HD))

    with tile.TileContext(nc) as tc:
        with tc.tile_pool(name="consts", bufs=1) as cp:
            ones_row_bf = cp.tile([1, 512], BF16)
            nc.vector.memset(ones_row_bf, 1.0)
            ones_col_bf = cp.tile([128, 1], BF16)
            nc.vector.memset(ones_col_bf, 1.0)
            eps_vec = cp.tile([1, 1], F32)
            nc.vector.memset(eps_vec, LN_EPS)
            mask_f = cp.tile([128, 16], F32)
            nc.gpsimd.dma_start(mask_f, dt_in["mask01f"][:])
            mask_bf = cp.tile([128, 16], BF16)
            nc.gpsimd.dma_start(mask_bf, dt_in["mask01bf"][:])

            _gnn(nc, tc, dt_in, xA, xAbf, ones_row_bf)
            if STAGES >= 2:
                _layer(nc, tc, dt_in, 0, xA, xAbf, ao_d, x1f_d, x1bf_d,
                       xB if STAGES >= 3 else out_stream, xBbf,
                       ones_row_bf, ones_col_bf, eps_vec, mask_f, mask_bf,
                       final=(STAGES == 2))
            if STAGES >= 3:
                _layer(nc, tc, dt_in, 1, xB, xBbf, ao_d, x1f_d, x1bf_d,
                       out_stream, None,
                       ones_row_bf, ones_col_bf, eps_vec, mask_f, mask_bf,
                       final=True)

    nc.compile()
    return nc


def _gnn(nc, tc, dt_in, xA, xAbf, ones_row_bf):
    """nodes = relu(X*r); x1 = relu(nodes@Wv1T+bv1); x2 = relu(x1@Wv2T+bv2);
    write stream tiles (m' = j*128+f layout) + positional encoding tile."""
    with tc.tile_pool(name="g_x1", bufs=1) as gx1:
        x1 = gx1.tile([128, KT_GNN * 1024], BF16)
        with (
            tc.tile_pool(name="g_in", bufs=1) as gin,
            tc.tile_pool(name="g_nodes", bufs=1) as gn,
            tc.tile_pool(name="g_tmp", bufs=3) as gtmp,
            tc.tile_pool(name="g_wv1", bufs=1) as gw1,
        ):
            xt_sb = gin.tile([128, 2 * 1024], BF16)
            nc.gpsimd.dma_start(xt_sb[:, 0:1024], dt_in["xt"][0:128, :])
            nc.gpsimd.dma_start(xt_sb[:, 1024:2048], dt_in["xt"][128:256, :])
            rrep = gin.tile([128, 8 * 1024], BF16)
            for j in range(8):
                src = dt_in["r8"][j:j+1, :]
                nc.gpsimd.dma_start(
                    rrep[:, j*1024:(j+1)*1024],
                    bass.AP(tensor=src.tensor, offset=src.offset, ap=[[0, 128], src.ap[-1]]))

            nodes = gn.tile([128, KT_GNN * 1024], BF16)    # tile t=2j+h at cols t*1024
            for t in range(KT_GNN):
                j, h = t // 2, t % 2
                tmp = gtmp.tile([128, 1024], BF16, tag="gt")
                nc.vector.tensor_mul(tmp, xt_sb[:, h*1024:(h+1)*1024], rrep[:, j*1024:(j+1)*1024])
                nc.vector.tensor_scalar_max(nodes[:, t*1024:(t+1)*1024], tmp, 0.0)

            wv1 = gw1.tile([128, KT_GNN * 2048], BF16)
            for kt in range(KT_GNN):
                nc.gpsimd.dma_start(wv1[:, kt*2048:(kt+1)*2048], dt_in["wv1t"][kt])
            bv1 = gw1.tile([128, 16], F32)
            nc.gpsimd.dma_start(bv1, dt_in["bv1pp"][:])

            with tc.tile_pool(name="g_ps1", bufs=2, space="PSUM") as ps1:
                for mt in range(KT_GNN):
                    pm = ps1.tile([128, 1024], F32)
                    for kt in range(KT_GNN):
                        lhs = wv1[:, kt*2048 + mt*128: kt*2048 + (mt+1)*128]
                        for nh in range(2):
                            nc.tensor.matmul(
                                pm[:, nh*512:(nh+1)*512], lhs,
                                nodes[:, kt*1024 + nh*512: kt*1024 + (nh+1)*512],
                                start=(kt == 0), stop=(kt == KT_GNN - 1))
                    nc.scalar.activation(x1[:, mt*1024:(mt+1)*1024], pm, AF.Relu,
                                         bias=bv1[:, mt:mt+1])

        # positional encoding -> stream tile 8 (rows 0..16)
        with tc.tile_pool(name="g_pe", bufs=2) as gpe:
            th_s = gpe.tile([8, T], F32, tag="th")
            th_c = gpe.tile([8, T], F32, tag="th")
            nc.gpsimd.dma_start(th_s, dt_in["thsin"][:])
            nc.gpsimd.dma_start(th_c, dt_in["thcos"][:])
            for half, th in ((0, th_s), (1, th_c)):
                pef = gpe.tile([8, T], F32, tag="pef")
                nc.scalar.activation(pef, th, AF.Sin)
                peb = gpe.tile([8, T], BF16, tag="peb")
                nc.vector.tensor_copy(peb, pef)
                nc.gpsimd.dma_start(xA[8, half*8:(half+1)*8, :], pef)
                nc.gpsimd.dma_start(xAbf[8, half*8:(half+1)*8, :], peb)

        # m2: lhsT = x1 slices (stationary), rhs = wv2 -> out [b-tile(f), d2'']
        with (
            tc.tile_pool(name="g_wv2", bufs=1) as gw2,
            tc.tile_pool(name="g_st", bufs=3) as gst,
            tc.tile_pool(name="g_ps2", bufs=6, space="PSUM") as ps2,
        ):
            wv2 = gw2.tile([128, KT_GNN * 2048], BF16)
            for kt in range(KT_GNN):
                nc.gpsimd.dma_start(wv2[:, kt*2048:(kt+1)*2048], dt_in["wv2t"][kt])
            bv2r = gw2.tile([1, 2048], BF16)
            nc.gpsimd.dma_start(bv2r, dt_in["bv2row"][:])

            for b in range(BL):
                for c4 in range(4):
                    pm = ps2.tile([128, 512], F32, tag="g2")
                    for kt in range(KT_GNN):
                        nc.tensor.matmul(
                            pm, x1[:, kt*1024 + b*128: kt*1024 + (b+1)*128],
                            wv2[:, kt*2048 + c4*512: kt*2048 + (c4+1)*512],
                            start=(kt == 0), stop=False)
                    nc.tensor.matmul(pm, ones_row_bf[0:1, 0:128],
                                     bv2r[:, c4*512:(c4+1)*512], start=False, stop=True)
                    stf = gst.tile([128, 512], F32, tag="gsf")
                    nc.scalar.activation(stf, pm, AF.Relu)
                    stb = gst.tile([128, 512], BF16, tag="gsb")
                    nc.scalar.activation(stb, pm, AF.Relu)
                    for jr in range(2):
                        tile_j = 2 * c4 + jr
                        nc.gpsimd.dma_start(xA[tile_j, :, b*256:(b+1)*256],
                                            stf[:, jr*256:(jr+1)*256])
                        nc.gpsimd.dma_start(xAbf[tile_j, :, b*256:(b+1)*256],
                                            stb[:, jr*256:(jr+1)*256])


def _layer(nc, tc, dt_in, li, x_in, x_in_bf, ao_d, x1f_d, x1bf_d, x_out, x_out_bf,
           ones_row_bf, ones_col_bf, eps_vec, mask_f, mask_bf, final):
    inv_sqrt_hd = 1.0 / float(np.sqrt(HD))
    KD = {8: 16}  # last m'-ktile has 16 rows

    def ktn(kt):
        return 16 if kt == 8 else 128

    # ---------------- P1a: QKV + attention -> ao_d ----------------
    with (
        tc.tile_pool(name=f"w_qkv{li}", bufs=1) as wp,
        tc.tile_pool(name=f"a_act{li}", bufs=2) as ap,
        tc.tile_pool(name=f"a_small{li}", bufs=3) as sp_,
        tc.tile_pool(name=f"a_x0{li}", bufs=2) as xp,
    ):
        wq = wp.tile([128, NMT * 1024], BF16)
        wk = wp.tile([128, NMT * 1024], BF16)
        wv_ = wp.tile([128, NMT * D], BF16)
        wql = wp.tile([128, NMT * 16], BF16)
        wkl = wp.tile([128, NMT * 16], BF16)
        for kt in range(NMT):
            nc.gpsimd.dma_start(wq[:, kt*1024:(kt+1)*1024], dt_in[f"wq{li}"][kt])
            nc.gpsimd.dma_start(wk[:, kt*1024:(kt+1)*1024], dt_in[f"wk{li}"][kt])
            nc.gpsimd.dma_start(wv_[:, kt*D:(kt+1)*D], dt_in[f"wv{li}"][kt])
            nc.gpsimd.dma_start(wql[:, kt*16:(kt+1)*16], dt_in[f"wql{li}"][kt])
            nc.gpsimd.dma_start(wkl[:, kt*16:(kt+1)*16], dt_in[f"wkl{li}"][kt])
        bq = wp.tile([128, 8], F32)
        bk = wp.tile([128, 8], F32)
        bql = wp.tile([16, 1], F32)
        bkl = wp.tile([16, 1], F32)
        bvr = wp.tile([1, D], BF16)
        nc.gpsimd.dma_start(bq, dt_in[f"bq{li}"][:])
        nc.gpsimd.dma_start(bk, dt_in[f"bk{li}"][:])
        nc.gpsimd.dma_start(bql, dt_in[f"bql{li}"][:])
        nc.gpsimd.dma_start(bkl, dt_in[f"bkl{li}"][:])
        nc.gpsimd.dma_start(bvr, dt_in[f"bvrow{li}"][:])

        for ch in range(4):                      # 512-token chunks = b-pairs
            t0 = ch * 512
            x0c = xp.tile([128, NMT * 512], BF16, tag="x0c")
            for kt in range(NMT):
                nc.gpsimd.dma_start(x0c[:ktn(kt), kt*512:(kt+1)*512],
                                    x_in_bf[kt, 0:ktn(kt), t0:t0+512])

            qm = ap.tile([128, 8 * 512], BF16, tag="qm")
            km = ap.tile([128, 8 * 512], BF16, tag="km")
            qlr = ap.tile([128, 4 * 512], BF16, tag="qlr")
            klr = ap.tile([128, 4 * 512], BF16, tag="klr")
            vv = ap.tile([128, 4 * D], BF16, tag="vv")

            with tc.tile_pool(name=f"ps_qkv{li}", bufs=3, space="PSUM") as pqk:
                for dst, w, bias in ((qm, wq, bq), (km, wk, bk)):
                    for qt in range(8):
                        pq = pqk.tile([128, 512], F32, tag="pq")
                        for kt in range(NMT):
                            nc.tensor.matmul(
                                pq, w[:ktn(kt), kt*1024 + qt*128: kt*1024 + (qt+1)*128],
                                x0c[:ktn(kt), kt*512:(kt+1)*512],
                                start=(kt == 0), stop=(kt == NMT - 1))
                        nc.scalar.activation(dst[:, qt*512:(qt+1)*512], pq, AF.Identity,
                                             bias=bias[:, qt:qt+1])
                # leftovers -> staging -> spread via DMA
                for dst, w, bias in ((qlr, wql, bql), (klr, wkl, bkl)):
                    pl = pqk.tile([16, 512], F32, tag="pq")
                    for kt in range(NMT):
                        nc.tensor.matmul(pl, w[:ktn(kt), kt*16:(kt+1)*16],
                                         x0c[:ktn(kt), kt*512:(kt+1)*512],
                                         start=(kt == 0), stop=(kt == NMT - 1))
                    stg = sp_.tile([16, 512], BF16, tag="stg")
                    nc.scalar.activation(stg, pl, AF.Identity, bias=bias)
                    for g in range(4):
                        d = dst[:, g*512:(g+1)*512]
                        nc.gpsimd.dma_start(
                            bass.AP(tensor=d.tensor, offset=d.offset,
                                    ap=[[32 * d.ap[0][0], 4], d.ap[-1]]),
                            stg[g*4:(g+1)*4, :])
                # V (token-stationary): out [token 128, ev chunk]
                for tt in range(4):
                    ttg = 4 * ch + tt
                    for ec, (e0, en) in enumerate(((0, 512), (512, 512), (1024, 16))):
                        pv = pqk.tile([128, 512], F32, tag="pq")
                        for kt in range(NMT):
                            nc.tensor.matmul(
                                pv[:, 0:en],
                                x0c[:ktn(kt), kt*512 + tt*128: kt*512 + (tt+1)*128],
                                wv_[:ktn(kt), kt*D + e0: kt*D + e0 + en],
                                start=(kt == 0), stop=False)
                        nc.tensor.matmul(pv[:, 0:en], ones_row_bf[0:1, 0:128],
                                         bvr[:, e0:e0+en], start=False, stop=True)
                        nc.scalar.activation(vv[:, tt*D + e0: tt*D + e0 + en],
                                             pv[:, 0:en], AF.Identity,
                                             scale=mask_f[:, ttg:ttg+1])

                # attention for the two b in this chunk
                with (
                    tc.tile_pool(name=f"ps_sp{li}", bufs=2, space="PSUM") as psp,
                    tc.tile_pool(name=f"ps_att{li}", bufs=1, space="PSUM") as pat,
                ):
                    for br in range(2):
                        b = 2 * ch + br
                        ao_b = sp_.tile([65, 16 * 256], BF16, tag="aob")
                        for h in range(16):
                            qt, qof = h // 2, 64 * (h % 2)
                            gt, gof = h // 4, 32 * (h % 4)
                            spt = psp.tile([128, 512], F32, tag="spt")
                            for lk2 in range(2):
                                nc.tensor.matmul(
                                    spt[:, lk2*256:(lk2+1)*256],
                                    km[qof:qof+64, qt*512 + br*256 + lk2*128:
                                       qt*512 + br*256 + (lk2+1)*128],
                                    qm[qof:qof+64, qt*512 + br*256: qt*512 + (br+1)*256],
                                    start=True, stop=False)
                                nc.tensor.matmul(
                                    spt[:, lk2*256:(lk2+1)*256],
                                    klr[gof:gof+1, gt*512 + br*256 + lk2*128:
                                        gt*512 + br*256 + (lk2+1)*128],
                                    qlr[gof:gof+1, gt*512 + br*256: gt*512 + (br+1)*256],
                                    start=False, stop=True, tile_position=(gof, 0))
                            ex = sp_.tile([128, 512], BF16, tag="ex")
                            nc.scalar.activation(ex, spt, AF.Exp, scale=inv_sqrt_hd)
                            dp = pat.tile([1, 256], F32, tag="dp")
                            av = pat.tile([65, 256], F32, tag="av")
                            for lk2 in range(2):
                                ttg = 4 * ch + 2 * br + lk2
                                nc.tensor.matmul(dp, mask_bf[:, ttg:ttg+1],
                                                 ex[:, lk2*256:(lk2+1)*256],
                                                 start=(lk2 == 0), stop=(lk2 == 1))
                                nc.tensor.matmul(
                                    av, vv[:, (2*br+lk2)*D + h*65: (2*br+lk2)*D + (h+1)*65],
                                    ex[:, lk2*256:(lk2+1)*256],
                                    start=(lk2 == 0), stop=(lk2 == 1))
                            rcpf = sp_.tile([1, 256], F32, tag="rcpf")
                            nc.vector.reciprocal(rcpf, dp)
                            rcp = sp_.tile([1, 256], BF16, tag="rcp")
                            nc.vector.tensor_copy(rcp, rcpf)
                            rp = pat.tile([65, 256], F32, tag="rp")
                            nc.tensor.matmul(rp, ones_row_bf[0:1, 0:65], rcp,
                                             start=True, stop=True)
                            rps = sp_.tile([65, 256], F32, tag="rps")
                            nc.scalar.activation(rps, rp, AF.Copy)
                            nc.vector.tensor_mul(
                                ao_b[:, h*256:(h+1)*256], av, rps)
                        for hh in range(16):
                            nc.gpsimd.dma_start(ao_d[hh, :, b*256:(b+1)*256],
                                                ao_b[:, hh*256:(hh+1)*256])

    # ---------------- P1b: Wo + residual + LN1 -> x1f_d / x1bf_d ----------------
    _proj_ln(nc, tc, dt_in, li, src_d=ao_d, src_tiles=16, src_w=f"wo{li}",
             src_brow=f"borow{li}", res_d=x_in, dst_f=x1f_d, dst_bf=x1bf_d,
             ln="1", ones_row_bf=ones_row_bf, ones_col_bf=ones_col_bf,
             eps_vec=eps_vec, relu=False, src_width=D, kpart=65)

    # ---------------- P2: FFN + LN2 -> x_out ----------------
    with (
        tc.tile_pool(name=f"w_ffn{li}", bufs=1) as wfp,
        tc.tile_pool(name=f"f_act{li}", bufs=2) as fap,
    ):
        wf1 = wfp.tile([128, NMT * D_FFN], BF16)
        for kt in range(NMT):
            nc.gpsimd.dma_start(wf1[:, kt*D_FFN:(kt+1)*D_FFN], dt_in[f"wf1{li}"][kt])
        bf1 = wfp.tile([128, 16], F32)
        nc.gpsimd.dma_start(bf1, dt_in[f"bf1pp{li}"][:])
        wf2 = wfp.tile([128, 16 * D], BF16)
        for kt in range(16):
            nc.gpsimd.dma_start(wf2[:, kt*D:(kt+1)*D], dt_in[f"wf2{li}"][kt])
        bf2r = wfp.tile([1, D], BF16)
        nc.gpsimd.dma_start(bf2r, dt_in[f"bf2row{li}"][:])

        fhp_cm = tc.tile_pool(name=f"f_h{li}", bufs=1)
        fhp = fhp_cm.__enter__()
        for ch in range(4):
            t0 = ch * 512
            x1c = fap.tile([128, NMT * 512], BF16, tag="x1c")
            for kt in range(NMT):
                nc.gpsimd.dma_start(x1c[:ktn(kt), kt*512:(kt+1)*512],
                                    x1bf_d[kt, 0:ktn(kt), t0:t0+512])
            hbuf = fhp.tile([128, 16 * 512], BF16, tag="hbuf")
            with tc.tile_pool(name=f"ps_f{li}", bufs=3, space="PSUM") as psf:
                for ht in range(16):
                    fp = psf.tile([128, 512], F32, tag="pp")
                    for kt in range(NMT):
                        nc.tensor.matmul(
                            fp, wf1[:ktn(kt), kt*D_FFN + ht*128: kt*D_FFN + (ht+1)*128],
                            x1c[:ktn(kt), kt*512:(kt+1)*512],
                            start=(kt == 0), stop=(kt == NMT - 1))
                    nc.scalar.activation(hbuf[:, ht*512:(ht+1)*512], fp, AF.Relu,
                                         bias=bf1[:, ht:ht+1])
                _proj_ln_chunk(nc, tc, dt_in, li, ch, src_sb=hbuf, src_tiles=16,
                               w_sb=wf2, brow_sb=bf2r, res_d=x1f_d,
                               dst_f=x_out, dst_bf=x_out_bf, ln="2",
                               ones_row_bf=ones_row_bf, ones_col_bf=ones_col_bf,
                               eps_vec=eps_vec, psum_pool=psf, act_pool=fap,
                               src_width=512)
        fhp_cm.__exit__(None, None, None)


def _proj_ln(nc, tc, dt_in, li, src_d, src_tiles, src_w, src_brow, res_d,
             dst_f, dst_bf, ln, ones_row_bf, ones_col_bf, eps_vec, relu, src_width,
             kpart=128):
    """Projection (contract src_tiles k-tiles) + residual + layernorm, streaming
    512-token chunks from DRAM. Used for Wo+LN1."""
    with (
        tc.tile_pool(name=f"w_pr{li}{ln}", bufs=1) as wp,
        tc.tile_pool(name=f"p_act{li}{ln}", bufs=2) as ap,
    ):
        w_sb = wp.tile([128, src_tiles * D], BF16)
        for kt in range(src_tiles):
            nc.gpsimd.dma_start(w_sb[:, kt*D:(kt+1)*D], dt_in[src_w][kt])
        brow = wp.tile([1, D], BF16)
        nc.gpsimd.dma_start(brow, dt_in[src_brow][:])

        for ch in range(4):
            t0 = ch * 512
            src_c = ap.tile([128, src_tiles * 512], BF16, tag="srcc")
            for kt in range(src_tiles):
                nc.gpsimd.dma_start(src_c[:kpart, kt*512:(kt+1)*512],
                                    src_d[kt, 0:kpart, t0:t0+512])
            with tc.tile_pool(name=f"ps_pr{li}{ln}", bufs=3, space="PSUM") as psp:
                _proj_ln_chunk(nc, tc, dt_in, li, ch, src_sb=src_c,
                               src_tiles=src_tiles, w_sb=w_sb, brow_sb=brow,
                               res_d=res_d, dst_f=dst_f, dst_bf=dst_bf, ln=ln,
                               ones_row_bf=ones_row_bf, ones_col_bf=ones_col_bf,
                               eps_vec=eps_vec, psum_pool=psp, act_pool=ap,
                               src_width=512, kpart=kpart)


def _proj_ln_chunk(nc, tc, dt_in, li, ch, src_sb, src_tiles, w_sb, brow_sb, res_d,
                   dst_f, dst_bf, ln, ones_row_bf, ones_col_bf, eps_vec,
                   psum_pool, act_pool, src_width, kpart=128):
    """One 512-token chunk: out = W @ src + brow + residual; then LN -> dst."""
    t0 = ch * 512
    ktn = lambda kt: kpart
    u = act_pool.tile([128, NMT * 512], BF16, tag=f"u{ln}")
    with tc.tile_pool(name=f"ps_st{li}{ln}{ch}", bufs=1, space="PSUM") as pst:
        ssum = pst.tile([1, 512], F32, tag="ss")
        ssq = pst.tile([1, 512], F32, tag="sq")
        for mt in range(NMT):
            n = 128 if mt < 8 else 16
            pp = psum_pool.tile([128, 512], F32, tag="pp")
            for kt in range(src_tiles):
                nc.tensor.matmul(
                    pp[:n, :],
                    w_sb[:ktn(kt), kt*D + mt*128: kt*D + mt*128 + n],
                    src_sb[:ktn(kt), kt*src_width:(kt+1)*src_width],
                    start=(kt == 0), stop=False)
            nc.tensor.matmul(pp[:n, :], brow_sb[0:1, mt*128:mt*128+n],
                             ones_row_bf[0:1, 0:512], start=False, stop=True)
            xr = act_pool.tile([128, 512], F32, tag=f"xr{ln}")
            nc.gpsimd.dma_start(xr[:n, :], res_d[mt, 0:n, t0:t0+512])
            nc.vector.tensor_add(u[:n, mt*512:(mt+1)*512], pp[:n, :], xr[:n, :])
            u2t = act_pool.tile([128, 512], BF16, tag=f"u2t{ln}")
            nc.vector.tensor_mul(u2t[:n, :],
                                 u[:n, mt*512:(mt+1)*512], u[:n, mt*512:(mt+1)*512])
            nc.tensor.matmul(ssum, ones_col_bf[:n, :], u[:n, mt*512:(mt+1)*512],
                             start=(mt == 0), stop=(mt == NMT - 1))
            nc.tensor.matmul(ssq, ones_col_bf[:n, :], u2t[:n, :],
                             start=(mt == 0), stop=(mt == NMT - 1))

        mu = act_pool.tile([1, 512], F32, tag=f"mu{ln}")
        nc.scalar.activation(mu, ssum, AF.Identity, scale=1.0 / D)
        mu2 = act_pool.tile([1, 512], F32, tag=f"sc{ln}")
        nc.vector.tensor_mul(mu2, mu, mu)
        var = act_pool.tile([1, 512], F32, tag=f"sc{ln}")
        nc.vector.scalar_tensor_tensor(var, ssq, 1.0 / D, mu2, op0=MUL,
                                       op1=mybir.AluOpType.subtract)
        sd = act_pool.tile([1, 512], F32, tag=f"sc{ln}")
        nc.scalar.activation(sd, var, AF.Sqrt, bias=eps_vec)
        ia = act_pool.tile([1, 512], F32, tag=f"ia{ln}")
        nc.vector.reciprocal(ia, sd)
        ib = act_pool.tile([1, 512], F32, tag=f"ib{ln}")
        nc.vector.scalar_tensor_tensor(ib, mu, -1.0, ia, op0=MUL, op1=MUL)
        ib_bf = act_pool.tile([1, 512], BF16, tag=f"ibb{ln}")
        nc.vector.tensor_copy(ib_bf, ib)

        ia_bf = act_pool.tile([1, 512], BF16, tag=f"iab{ln}")
        nc.vector.tensor_copy(ia_bf, ia)
        arep = pst.tile([128, 512], F32, tag="ar")
        nc.tensor.matmul(arep, ones_row_bf[0:1, 0:128], ia_bf, start=True, stop=True)

        s_pp = act_pool.tile([128, NMT], F32, tag=f"spp{ln}")
        b_pp = act_pool.tile([128, NMT], F32, tag=f"bpp{ln}")
        nc.gpsimd.dma_start(s_pp, dt_in[f"ln{ln}s{li}"][:])
        nc.gpsimd.dma_start(b_pp, dt_in[f"ln{ln}b{li}"][:])
        srow = act_pool.tile([1, D], BF16, tag=f"srw{ln}")
        brw = act_pool.tile([1, D], BF16, tag=f"brw{ln}")
        nc.gpsimd.dma_start(srow, dt_in[f"ln{ln}srow{li}"][:])
        nc.gpsimd.dma_start(brw, dt_in[f"ln{ln}brow{li}"][:])

        for mt in range(NMT):
            n = 128 if mt < 8 else 16
            bsp = psum_pool.tile([128, 512], F32, tag="pp")
            nc.tensor.matmul(bsp[:n, :], srow[0:1, mt*128:mt*128+n], ib_bf,
                             start=True, stop=False)
            nc.tensor.matmul(bsp[:n, :], brw[0:1, mt*128:mt*128+n],
                             ones_row_bf[0:1, 0:512], start=False, stop=True)
            t1 = act_pool.tile([128, 512], F32, tag=f"t1{ln}")
            nc.vector.scalar_tensor_tensor(t1[:n, :], u[:n, mt*512:(mt+1)*512],
                                           s_pp[:n, mt:mt+1], arep[:n, :],
                                           op0=MUL, op1=MUL)
            yf = act_pool.tile([128, 512], F32, tag=f"yf{ln}")
            nc.vector.tensor_add(yf[:n, :], t1[:n, :], bsp[:n, :])
            nc.gpsimd.dma_start(dst_f[mt, 0:n, t0:t0+512], yf[:n, :])
            if dst_bf is not None:
                yb = act_pool.tile([128, 512], BF16, tag=f"yb{ln}")
                nc.scalar.activation(yb[:n, :], yf[:n, :], AF.Copy)
                nc.gpsimd.dma_start(dst_bf[mt, 0:n, t0:t0+512], yb[:n, :])


# ---------------------------------------------------------------------------
# host entry
# ---------------------------------------------------------------------------

_CACHE = {}


def kernel(**inputs):
    X = np.asarray(inputs["X"])
    timestamps = np.asarray(inputs["timestamps"])
    lengths = np.asarray(inputs["lengths"])

    if "nc" not in _CACHE:
        _CACHE["nc"] = build_nc()
    nc = _CACHE["nc"]

    wprep = prep_weights(inputs)
    in_maps = []
    for core in range(NCORES):
        m = dict(wprep)
        m.update(prep_core_inputs(X, timestamps, lengths, core))
        in_maps.append(m)

    res = run_bass_kernel_spmd(nc, in_maps, core_ids=list(range(NCORES)))

    out = np.empty((L, B, D), np.float32)
    # stream row m' -> original d: build gather index [d] -> (tile, row)
    inv = np.empty(D, np.int64)
    inv[PERM_M] = np.arange(D)  # original d -> m'
    tiles, rows = inv // 128, inv % 128
    for core in range(NCORES):
        arr = res.results[core]["out_stream"]          # [9, 128, 2048]
        feat = arr[tiles, rows]                        # [D, 2048]
        out[:, core*BL:(core+1)*BL, :] = (
            feat.reshape(D, BL, L).transpose(2, 1, 0))
    lens = lengths.reshape(B).astype(np.int64)
    mask = np.arange(L)[None, :] >= lens[:, None]
    return out, mask


if __name__ == "__main__":
    import reference
    ins = {k: np.asarray(v) for k, v in reference.setup_inputs().items()}
    got, gmask = kernel(**ins)
    want, wmask = reference.reference(**ins)
    want = np.asarray(want)
    err = np.abs(got - want).max()
    print("maxabs", err, "rel", err / np.abs(want).max())
    print("mask ok:", bool((gmask == np.asarray(wmask)).all()))
